# revision 1
# baseline (speedup 1.0000x reference)
"""Cross-attention kernel for Trainium2, 8 NeuronCores.

Reference computation (B=4, S=2048, C=1024, E=1024, D=768, H=16, hd=64):
    q = x @ q_w + q_b                 # [B,S,E]
    k = context @ k_w + k_b           # [B,C,E]
    v = context @ v_w + v_b           # [B,C,E]
    attn = softmax(q.k^T / sqrt(hd))  # per head
    out = (attn @ v) @ o_w + o_b      # [B,S,E]

Sharding: 8 cores = 4 batches x 2 head-groups (8 heads = 512 embed cols each).
Each core computes the full attention for its (batch, head-group) and a
partial out-projection; the host sums the two head-group partials per batch
(the "all-reduce") and adds o_b.

Device layout: everything is computed in a transposed orientation so no
on-device transposes are needed.  The host passes x^T and context^T; the
projections produce Q^T/K^T with the head dim on partitions and V in natural
layout.  Scores are computed transposed (S^T = K @ Q^T, contraction over
hd=64, two heads packed into the 128-row PE array via row groups), the
softmax denominator comes free from the attention@V matmul by appending a
ones column to V (stationary operand is [V_h | 1], M=65), and the final
normalization is a per-column multiply using a gpsimd partition-broadcast of
the reciprocal sums.  All matmuls run as float32r (fp22 multiply, fp32
accumulate) which is full-rate on the PE for 512-wide moving operands.

The attention inner loop is ACT-bound (two exps of [128,512] per c-step vs
three matmul-slots of PE work), so the emission is software-pipelined: the
Q-projection matmuls for s-tile n+1 and the out-projection matmuls for
s-tile n-1 are interleaved into attention(n)'s c-steps to keep the PE fed
while the scalar engine works through the exps.
"""

import sys

sys.path.insert(0, "/opt/trn_rl_repo")

import numpy as np

B, S, E, C, D = 4, 2048, 1024, 1024, 768
H, HD = 16, 64
EL = E // 2          # embed columns per head-group (8 heads)
N_CORES = 8
NS = S // 512        # s-tiles of 512
KE = E // 128        # contraction chunks for q-proj
KD = D // 128        # contraction chunks for k/v-proj
NC2 = C // 512       # c-tiles of 512
CC = C // 128        # c chunks of 128
HP = EL // 128       # head pairs per core (4)

# "fp32r" (fp22 multiply, ~2.8e-4 end-to-end rel err) or "fp16"
# (halves DMA traffic and SBUF, ~1e-3 rel err)
DTYPE_MODE = "fp32r"

_built = None
_last_results = None


def _build(reps=1, nop_us=0, mode=None):
    import concourse.bacc as bacc
    import concourse.mybir as mybir
    from concourse.tile import TileContext

    F32 = mybir.dt.float32
    F32R = mybir.dt.float32r
    F16 = mybir.dt.float16
    Exp = mybir.ActivationFunctionType.Exp
    Ident = mybir.ActivationFunctionType.Identity

    if mode is None:
        mode = DTYPE_MODE
    CT = F32R if mode == "fp32r" else F16   # compute dtype for matmul operands
    IN = F32 if mode == "fp32r" else F16    # dram dtype for matmul inputs

    nc = bacc.Bacc(None, target_bir_lowering=False)

    xT = nc.declare_dram_parameter("xT", [E, S], IN, isOutput=False)
    ctxT = nc.declare_dram_parameter("ctxT", [D, C], IN, isOutput=False)
    qw = nc.declare_dram_parameter("qw", [E, EL], IN, isOutput=False)
    kw = nc.declare_dram_parameter("kw", [D, EL], IN, isOutput=False)
    vw = nc.declare_dram_parameter("vw", [D, EL], IN, isOutput=False)
    ow = nc.declare_dram_parameter("ow", [EL, E], IN, isOutput=False)
    qb = nc.declare_dram_parameter("qb", [EL, 1], F32, isOutput=False)
    kb = nc.declare_dram_parameter("kb", [EL, 1], F32, isOutput=False)
    vb = nc.declare_dram_parameter("vb", [1, EL], IN, isOutput=False)
    ones_r = nc.declare_dram_parameter("ones_r", [1, 128], IN, isOutput=False)
    out = nc.declare_dram_parameter("out", [S, E], F32, isOutput=True)

    def r(ap):
        return ap.bitcast(F32R) if mode == "fp32r" else ap

    with TileContext(nc) as tc:
        with (
            tc.tile_pool(name="wpool", bufs=1) as wpool,
            tc.tile_pool(name="dpool", bufs=1) as dpool,
            tc.tile_pool(name="xpool", bufs=4) as xpool,
            tc.tile_pool(name="qtpool", bufs=8) as qtpool,
            tc.tile_pool(name="ptpool", bufs=4) as ptpool,
            tc.tile_pool(name="otpool", bufs=8) as otpool,
            tc.tile_pool(name="spool", bufs=2) as spool,
            tc.tile_pool(name="opool", bufs=2) as opool,
            tc.tile_pool(name="pspool", bufs=1, space="PSUM") as pspool,
        ):
          for _rep in range(reps):
            # ---- weight / bias / context loads ---------------------------
            # One strided mega-DMA per tensor (chunks packed side-by-side in
            # a single SBUF tile, per-chunk views sliced out): each dma_start
            # holds the global HWDGE issue slot ~625ns, so fewer+bigger wins.
            # Ordered by first use: kw+ctx(first half) -> vw -> rest.
            def chunked_tile(pool, nchunk, width, name):
                t = pool.tile([128, nchunk * width], CT, name=name)
                return t, [t[:, i * width:(i + 1) * width] for i in range(nchunk)]

            # per-chunk DMAs for the prologue-critical tensors so the PE can
            # start as soon as the first chunks land
            _, kw_sb = chunked_tile(wpool, KD, EL, "kw_all")
            _, vw_sb = chunked_tile(wpool, KD, EL, "vw_all")
            ctx_all = dpool.tile([128, KD * C], CT, name="ctx_all")
            ctx_sb = [ctx_all[:, d * C:(d + 1) * C] for d in range(KD)]
            for d in range(KD):
                nc.sync.dma_start(
                    out=kw_sb[d][:], in_=r(kw[d * 128:(d + 1) * 128, :]))
                nc.sync.dma_start(
                    out=ctx_sb[d][:, 0:512],
                    in_=r(ctxT[d * 128:(d + 1) * 128, 0:512]))
            for d in range(KD):
                nc.sync.dma_start(
                    out=vw_sb[d][:], in_=r(vw[d * 128:(d + 1) * 128, :]))
            for d in range(KD):
                nc.sync.dma_start(
                    out=ctx_sb[d][:, 512:1024],
                    in_=r(ctxT[d * 128:(d + 1) * 128, 512:1024]))
            kb_t = wpool.tile([128, HP], F32, name="kb_t")
            nc.sync.dma_start(
                out=kb_t.rearrange("p (c w) -> p c w", w=1),
                in_=kb.rearrange("(c p) w -> p c w", p=128),
            )
            kb_sb = [kb_t[:, m:m + 1] for m in range(HP)]
            qb_t = wpool.tile([128, HP], F32, name="qb_t")
            nc.sync.dma_start(
                out=qb_t.rearrange("p (c w) -> p c w", w=1),
                in_=qb.rearrange("(c p) w -> p c w", p=128),
            )
            qb_sb = [qb_t[:, m:m + 1] for m in range(HP)]
            vb_sb = wpool.tile([1, EL], CT, name="vb_sb")
            nc.sync.dma_start(out=vb_sb[:], in_=r(vb[:]))
            ones_sb = wpool.tile([1, 128], CT, name="ones_sb")
            nc.sync.dma_start(out=ones_sb[:], in_=r(ones_r[:]))
            vb_bc = wpool.tile([128, EL], F32, name="vb_bc")
            vb_ps = pspool.tile([128, 512], F32, name="acc_ps", tag="acc", bufs=2)
            nc.tensor.matmul(vb_ps[:], ones_sb[0:1, :], vb_sb[:],
                             start=True, stop=True)
            nc.vector.tensor_copy(vb_bc[:], vb_ps[:])
            _, qw_sb = chunked_tile(wpool, KE, EL, "qw_all")
            for k in range(KE):
                nc.sync.dma_start(
                    out=qw_sb[k][:], in_=r(qw[k * 128:(k + 1) * 128, :]))
            ow_all = wpool.tile([128, HP * E], CT, name="ow_all")
            ow_sb = [ow_all[:, k * E:(k + 1) * E] for k in range(HP)]

            def load_ow():
                nc.sync.dma_start(
                    out=ow_all.rearrange("p (c w) -> p c w", w=E),
                    in_=r(ow).rearrange("(c p) w -> p c w", p=128),
                )

            # ---- K^T projection: [EL rows, C cols], head pairs on partitions --
            kt_sb = []
            for m in range(HP):
                t = dpool.tile([128, C], CT, name=f"kt{m}")
                kt_sb.append(t)

            def kt_thunks(m, t2s=range(NC2)):
                """Matmul thunks computing K^T halves for head pair m."""
                state = {}
                thunks = []

                def f(t2, d):
                    if d == 0:
                        state[t2] = pspool.tile(
                            [128, 512], F32, name="acc_ps", tag="acc", bufs=2)
                    ps = state[t2]
                    nc.tensor.matmul(
                        ps[:],
                        kw_sb[d][:, m * 128:(m + 1) * 128],
                        ctx_sb[d][:, t2 * 512:(t2 + 1) * 512],
                        start=(d == 0), stop=(d == KD - 1),
                    )
                    if d == KD - 1:
                        nc.vector.tensor_scalar_add(
                            kt_sb[m][:, t2 * 512:(t2 + 1) * 512], ps[:],
                            kb_sb[m][:, 0:1],
                        )

                for t2 in t2s:
                    for d in range(KD):
                        thunks.append((f, t2, d))
                return thunks

            # ---- V projection: natural [C rows, EL cols], interleaved with a
            # ones column per head for the softmax denominator ------------------
            v_sb = []
            for mc in range(CC):
                t = dpool.tile([128, 8 * 65], CT, name=f"v{mc}")
                v_sb.append(t)

            def vproj_group(mc):
                t = v_sb[mc]
                ps = pspool.tile([128, 512], F32, name="acc_ps", tag="acc", bufs=2)
                for d in range(KD):
                    nc.tensor.matmul(
                        ps[:],
                        ctx_sb[d][:, mc * 128:(mc + 1) * 128],
                        vw_sb[d][:],
                        start=(d == 0), stop=(d == KD - 1),
                    )
                vv = t.rearrange("p (h u) -> p h u", u=65)
                nc.vector.tensor_add(
                    vv[:, :, 0:64],
                    ps.rearrange("p (h u) -> p h u", u=64),
                    vb_bc.rearrange("p (h u) -> p h u", u=64),
                )
                nc.vector.tensor_scalar(
                    vv[:, :, 64:65],
                    vb_bc[:, 0:8].rearrange("p (h u) -> p h u", u=1),
                    0.0, 1.0,
                    mybir.AluOpType.mult, mybir.AluOpType.add,
                )  # writes the constant 1.0 column

            # ---- pipelined main loop over s-tiles of 512 ----------------------
            xts_all = {}
            qts_all = {}
            ots_all = {}

            def load_x(n):
                tiles = []
                for half in range(2):
                    t = xpool.tile([128, 4 * 512], CT, name="xt", tag="xt")
                    views = [t[:, i * 512:(i + 1) * 512] for i in range(4)]
                    if n == 0:
                        # n=0 is on the startup critical path: per-chunk DMAs
                        for i in range(4):
                            k = half * 4 + i
                            nc.sync.dma_start(
                                out=views[i][:],
                                in_=r(xT[k * 128:(k + 1) * 128,
                                         n * 512:(n + 1) * 512]))
                    else:
                        nc.sync.dma_start(
                            out=t.rearrange("p (c w) -> p c w", w=512),
                            in_=r(xT[half * 512:(half + 1) * 512,
                                     n * 512:(n + 1) * 512])
                            .rearrange("(c p) w -> p c w", p=128),
                        )
                    tiles += views
                xts_all[n] = tiles

            def qproj_thunks(n):
                """32 matmul thunks computing Q^T for s-tile n (4 psum groups)."""
                state = {}
                thunks = []
                qts_all[n] = [None] * HP

                def f(m, k):
                    if k == 0:
                        state[m] = pspool.tile(
                            [128, 512], F32, name="acc_ps", tag="acc", bufs=2)
                    ps = state[m]
                    nc.tensor.matmul(
                        ps[:],
                        qw_sb[k][:, m * 128:(m + 1) * 128],
                        xts_all[n][k][:],
                        start=(k == 0), stop=(k == KE - 1),
                    )
                    if k == KE - 1:
                        qt_t = qtpool.tile([128, 512], CT, name="qt", tag="qt")
                        nc.vector.tensor_scalar_add(qt_t[:], ps[:], qb_sb[m][:, 0:1])
                        qts_all[n][m] = qt_t

                for m in range(HP):
                    for k in range(KE):
                        thunks.append((f, m, k))
                return thunks

            def outproj_thunks(n):
                """32 matmul thunks for the out-projection of s-tile n."""
                state = {}
                thunks = []

                def f(ss, ne, hp):
                    if hp == 0:
                        state[(ss, ne)] = pspool.tile(
                            [128, 512], F32, name="acc_ps", tag="acc", bufs=2)
                        if ne == 0:
                            state[ss] = opool.tile(
                                [128, 1024], F32, name="o_sb", tag="o")
                    ps = state[(ss, ne)]
                    nc.tensor.matmul(
                        ps[:],
                        ots_all[n][hp][:, ss * 128:(ss + 1) * 128],
                        ow_sb[hp][:, ne * 512:(ne + 1) * 512],
                        start=(hp == 0), stop=(hp == HP - 1),
                    )
                    if hp == HP - 1:
                        o_sb = state[ss]
                        nc.vector.tensor_copy(
                            o_sb[:, ne * 512:(ne + 1) * 512], ps[:])
                        if ne == 1:
                            nc.sync.dma_start(
                                out=out[n * 512 + ss * 128:
                                        n * 512 + (ss + 1) * 128, :],
                                in_=o_sb[:],
                            )

                for ss in range(4):
                    for ne in range(2):
                        for hp in range(HP):
                            thunks.append((f, ss, ne, hp))
                return thunks

            def run_thunks(ts):
                for f, *args in ts:
                    f(*args)

            # prologue, ordered to match DMA arrival (kw+ctx.h1, vw, ctx.h2,
            # qw+xT0): K^T m=0 and V directly, then Q^T(0) m=0; the other head
            # pairs' K^T and Q^T groups ride in attention(0)'s background,
            # ordered so each lands before the head pair that needs it.
            load_x(0)
            load_ow()
            run_thunks(kt_thunks(0, t2s=[0]))
            for mc in range(4):
                vproj_group(mc)
            run_thunks(kt_thunks(0, t2s=[1]))
            for mc in range(4, CC):
                vproj_group(mc)
            qp0 = qproj_thunks(0)
            run_thunks(qp0[:KE])          # m=0 group
            prologue_bg = []
            for m in range(1, HP):
                prologue_bg += kt_thunks(m)
                prologue_bg += qp0[m * KE:(m + 1) * KE]

            for n in range(NS):
                if n + 1 < NS:
                    load_x(n + 1)
                bg = []
                if n == 0:
                    bg += prologue_bg
                if n + 1 < NS:
                    bg += qproj_thunks(n + 1)
                if n >= 1:
                    bg += outproj_thunks(n - 1)

                ots_all[n] = [None] * HP
                qts = qts_all[n]
                n_steps = HP * CC
                step = 0
                bg_done = 0
                for hp in range(HP):
                    ovs = [
                        pspool.tile([65, 512], F32, name="ov_ps", tag="ov", bufs=2)
                        for _ in range(2)
                    ]
                    for c in range(CC):
                        pts = []
                        for h2 in range(2):
                            sc = pspool.tile(
                                [128, 512], F32, name="sc_ps", tag="sc", bufs=3)
                            # scores^T block: K_h @ Q_h^T, contraction hd=64.
                            # h2=0 uses PE rows 0-63, h2=1 rows 64-127 -> the
                            # two matmuls run concurrently in row groups.
                            nc.tensor.matmul(
                                sc[:],
                                kt_sb[hp][h2 * 64:(h2 + 1) * 64,
                                          c * 128:(c + 1) * 128],
                                qts[hp][h2 * 64:(h2 + 1) * 64, :],
                                start=True, stop=True,
                            )
                            p = ptpool.tile([128, 512], CT, name="pt", tag="pt")
                            nc.scalar.activation(p[:], sc[:], Exp)
                            pts.append(p)
                        # inject background (q-proj n+1 / out-proj n-1) work
                        # between the scores and the exp-gated AV matmuls so
                        # the PE stays busy through the exp latency
                        step += 1
                        target = step * len(bg) // n_steps
                        while bg_done < target:
                            f, *args = bg[bg_done]
                            f(*args)
                            bg_done += 1
                        for h2 in range(2):
                            h = hp * 2 + h2
                            nc.tensor.matmul(
                                ovs[h2][:],
                                v_sb[c][:, h * 65:(h + 1) * 65],
                                pts[h2][:],
                                start=(c == 0), stop=(c == CC - 1),
                            )
                    # normalization epilogue for this head pair
                    ot_t = otpool.tile([128, 512], CT, name="ot", tag="ot")
                    for h2 in range(2):
                        rs = spool.tile([1, 512], CT, name="rs", tag="rs")
                        with nc.allow_low_precision("softmax denom, fp22 ok"):
                            nc.vector.reciprocal(rs[:], ovs[h2][64:65, :])
                        bc_ps = pspool.tile([64, 512], F32, name="bc_ps",
                                            tag="bc", bufs=1)
                        nc.tensor.matmul(bc_ps[:], ones_sb[0:1, 0:64], rs[:],
                                         start=True, stop=True)
                        bc = spool.tile([64, 512], F32, name="bc", tag="bc")
                        nc.vector.tensor_copy(bc[:], bc_ps[:])
                        nc.vector.tensor_mul(
                            ot_t[h2 * 64:(h2 + 1) * 64, :], ovs[h2][0:64, :], bc[:]
                        )
                    ots_all[n][hp] = ot_t
                run_thunks(bg[bg_done:])

            # epilogue: out-projection of the last s-tile
            run_thunks(outproj_thunks(NS - 1))

          # timing aid: calibrated delay chain on the otherwise-idle gpsimd
          # engine; kernel exec time = max(real work, nop chain)
          if nop_us:
            NOP_CYC = 48000  # 40 us at 1.2 GHz
            for _ in range(int(nop_us * 1200 / NOP_CYC)):
                nc.gpsimd.nop(cycle_cnt=NOP_CYC, nofuse=True)

    nc.finalize()
    return nc


def kernel(x, context, q_w, q_b, k_w, k_b, v_w, v_b, o_w, o_b):
    global _built, _last_results
    from concourse.bass_utils import run_bass_kernel_spmd

    if _built is None:
        _built = _build()
    nc = _built

    scale = np.float32(1.0 / np.sqrt(HD))
    ind = np.float32 if DTYPE_MODE == "fp32r" else np.float16
    x = np.asarray(x, np.float32)
    context = np.asarray(context, np.float32)
    xTs = [np.ascontiguousarray(x[b].T).astype(ind) for b in range(B)]
    ctxTs = [np.ascontiguousarray(context[b].T).astype(ind) for b in range(B)]

    in_maps = []
    for core in range(N_CORES):
        b, hg = core // 2, core % 2
        el = slice(hg * EL, (hg + 1) * EL)
        in_maps.append({
            "xT": xTs[b],
            "ctxT": ctxTs[b],
            "qw": np.ascontiguousarray(
                (np.asarray(q_w, np.float32)[:, el] * scale).astype(ind)),
            "kw": np.ascontiguousarray(np.asarray(k_w, np.float32)[:, el]).astype(ind),
            "vw": np.ascontiguousarray(np.asarray(v_w, np.float32)[:, el]).astype(ind),
            "ow": np.ascontiguousarray(np.asarray(o_w, np.float32)[el, :]).astype(ind),
            "qb": np.ascontiguousarray(
                (np.asarray(q_b, np.float32)[el] * scale)[:, None]),
            "kb": np.ascontiguousarray(np.asarray(k_b, np.float32)[el][:, None]),
            "vb": np.ascontiguousarray(
                np.asarray(v_b, np.float32)[el][None, :]).astype(ind),
            "ones_r": np.ones((1, 128), ind),
        })

    res = run_bass_kernel_spmd(nc, in_maps, list(range(N_CORES)))
    _last_results = res

    ob = np.asarray(o_b, np.float32)
    full = np.empty((B, S, E), np.float32)
    for b in range(B):
        full[b] = res.results[2 * b]["out"] + res.results[2 * b + 1]["out"] + ob
    return full



# revision 69
# speedup vs baseline: 1.2005x; 1.2005x over previous
"""Cross-attention kernel for Trainium2, 8 NeuronCores.

Reference computation (B=4, S=2048, C=1024, E=1024, D=768, H=16, hd=64):
    q = x @ q_w + q_b                 # [B,S,E]
    k = context @ k_w + k_b           # [B,C,E]
    v = context @ v_w + v_b           # [B,C,E]
    attn = softmax(q.k^T / sqrt(hd))  # per head
    out = (attn @ v) @ o_w + o_b      # [B,S,E]

Sharding: 8 cores = 4 batches x 2 head-groups (8 heads = 512 embed cols each).
Each core computes the full attention for its (batch, head-group) and a
partial out-projection; the host sums the two head-group partials per batch
(the "all-reduce") and adds o_b.

Per-core design (all matmul operands fp16; psum f32):
  - Projections as in the natural layouts: K^T [pair(2 heads)=128p, C],
    V [c=128p, 8 heads x 65] (65th column = ones for the softmax denom),
    Q^T [pair=128p, 512 s] per s-tile of 512.
  - Scores transposed: S^T[c,s] = K_h @ Q_h^T (contraction hd=64, two heads
    share the 128-row PE via row groups).  Each [c=128, s=512] score block
    fills one psum bank; its exp is a single 512-wide ACT instruction (the
    ACT engine charges per free-dim element; a wider activation would cross
    a psum bank boundary, which faults on hardware).
  - attention@V flipped: P^T[c, s-chunk 128] is the *stationary* operand and
    V[c, 65] the moving one -> 65-row matmuls, half the PE rows of the
    natural orientation.  The four s-chunk accumulators pack into one psum
    bank ([128, 260]; start=True only on the very first slice-group,
    stop=True only on the last, later slice-groups start on pending-zero
    bytes).  Output lands [s, el], so the softmax denominators sit on the
    s-partition axis: normalization is a batched DVE reciprocal plus
    per-partition tensor_scalar multiplies (no broadcast matmuls).
  - The [s, el] -> [el, s] transpose for the out-projection runs on the DMA
    XBAR (dma_start_transpose, fp16), off the PE entirely.
  - Software pipeline: the AV matmuls for step k-1 are emitted *after* the
    scores matmuls of step k, so the exp(k-1) latency hides behind scores(k)
    plus injected background work (q-proj of s-tile n+1, out-proj of s-tile
    n-1, K/V projection during the first tile).  The PE stream never waits
    on the ACT engine in steady state.

Cost model: PE ~378k rows ~= 158us busy; ACT 256 exps x 611ns ~= 158us busy;
steady-state s-tiles are ACT-bound (~39us each), s-tile 0 is PE-bound (the
K/V/Q projections ride its background), measured 201175 ns end-to-end.
"""

import os
import sys

sys.path.insert(0, "/opt/trn_rl_repo")

import numpy as np

# Debug toggles (defaults are the hardware-validated config; env overrides
# for bisection).  EXPW=1024 would read PSUM across a bank boundary in one
# activation — the cost model allows it but real hardware faults
# (NRT_EXEC_UNIT_UNRECOVERABLE), so the default must stay 512.
EXPW = int(os.environ.get("K_EXPW", "512"))       # exp width: 512 (1 bank) max
TR_MODE = os.environ.get("K_TR", "dma")           # attn transpose: "dma" or "pe"

B, S, E, C, D = 4, 2048, 1024, 1024, 768
H, HD = 16, 64
EL = E // 2          # embed columns per head-group (8 heads)
N_CORES = 8
NS = S // 512        # s-tiles of 512
KE = E // 128        # contraction chunks for q-proj
KD = D // 128        # contraction chunks for k/v-proj
CC = C // 128        # c chunks of 128
HP = EL // 128       # head pairs per core (4)
NH = 2 * HP          # heads per core (8)

_built = None
_last_results = None


def _build():
    import concourse.bacc as bacc
    import concourse.mybir as mybir
    from concourse.tile import TileContext

    F32 = mybir.dt.float32
    F16 = mybir.dt.float16
    Exp = mybir.ActivationFunctionType.Exp

    nc = bacc.Bacc(None, target_bir_lowering=False)

    xT = nc.declare_dram_parameter("xT", [E, S], F16, isOutput=False)
    ctxT = nc.declare_dram_parameter("ctxT", [D, C], F16, isOutput=False)
    qw = nc.declare_dram_parameter("qw", [E, EL], F16, isOutput=False)
    kw = nc.declare_dram_parameter("kw", [D, EL], F16, isOutput=False)
    vw = nc.declare_dram_parameter("vw", [D, EL], F16, isOutput=False)
    ow = nc.declare_dram_parameter("ow", [EL, E], F16, isOutput=False)
    kbqb = nc.declare_dram_parameter("kbqb", [EL, 2], F32, isOutput=False)
    vbo = nc.declare_dram_parameter("vbo", [1, EL + 128], F16, isOutput=False)
    ident = nc.declare_dram_parameter("ident", [128, 128], F16, isOutput=False)
    out = nc.declare_dram_parameter("out", [S, E], F16, isOutput=True)

    # psum bank budget (8 banks total)
    if EXPW == 1024:
        assert TR_MODE == "dma", "1024-wide exp + PE transpose exceeds psum"
        sc_bufs = 2          # 2 banks each -> 4
    else:
        sc_bufs = 4 if TR_MODE == "dma" else 3

    with TileContext(nc) as tc:
        with (
            tc.tile_pool(name="wpool", bufs=1) as wpool,
            tc.tile_pool(name="dpool", bufs=1) as dpool,
            tc.tile_pool(name="xpool", bufs=4) as xpool,
            tc.tile_pool(name="qtpool", bufs=8) as qtpool,
            tc.tile_pool(name="ptpool", bufs=3) as ptpool,
            tc.tile_pool(name="apool", bufs=8) as apool,
            tc.tile_pool(name="atpool", bufs=8) as atpool,
            tc.tile_pool(name="spool", bufs=3) as spool,
            tc.tile_pool(name="opool", bufs=2) as opool,
            tc.tile_pool(name="pspool", bufs=1, space="PSUM") as pspool,
        ):
            # ---- weight / bias / context loads ---------------------------
            # Ordered by first use on the PE: kw + ctx(first half) gate the
            # K-projection, then xT(0)+qw for the first Q-projection, then
            # vw, ctx(second half), ow.
            def chunked_tile(pool, nchunk, width, name):
                t = pool.tile([128, nchunk * width], F16, name=name)
                return t, [t[:, i * width:(i + 1) * width] for i in range(nchunk)]

            kw_all, kw_sb = chunked_tile(wpool, KD, EL, "kw_all")
            vw_all, vw_sb = chunked_tile(wpool, KD, EL, "vw_all")
            ctx_all = dpool.tile([128, KD * C], F16, name="ctx_all")
            ctx_sb = [ctx_all[:, d * C:(d + 1) * C] for d in range(KD)]

            # Batched mega-DMAs (the SP sequencer + global HWDGE issue slot
            # cost ~650ns per dma_start regardless of size; transfers
            # serialize on the DMA engines), ordered by the critical chain to
            # the first exp: biases (packed into 2 tiny params), kw+ctx(h0)
            # (gate K-proj), xT0+qw (gate Q-proj m0 / the first attention
            # step), then vw, ctx(h1), the other x tiles, ow — all consumed
            # via background work later in s-tile 0.
            kb_t = wpool.tile([128, 2 * HP], F32, name="kb_t")
            nc.sync.dma_start(
                out=kb_t.rearrange("p (c w) -> p c w", w=2),
                in_=kbqb.rearrange("(c p) w -> p c w", p=128),
            )
            kbv = kb_t.rearrange("p (c w) -> p c w", w=2)
            vbo_sb = wpool.tile([1, EL + 128], F16, name="vbo_sb")
            nc.sync.dma_start(out=vbo_sb[:], in_=vbo[:])
            vb_sb = vbo_sb[:, 0:EL]
            ones_sb = vbo_sb[:, EL:EL + 128]
            nc.sync.dma_start(
                out=kw_all.rearrange("p (c w) -> p c w", w=EL),
                in_=kw.rearrange("(c p) w -> p c w", p=128),
            )
            nc.sync.dma_start(
                out=ctx_all.rearrange("p (c w) -> p c w", w=C)[:, :, 0:512],
                in_=ctxT[:, 0:512].rearrange("(c p) w -> p c w", p=128),
            )

            _qw_tile, qw_sb = chunked_tile(wpool, KE, EL, "qw_all")
            xts_all = {}

            def load_x(n):
                t = xpool.tile([128, KE * 512], F16, name="xt", tag="xt")
                views = [t[:, i * 512:(i + 1) * 512] for i in range(KE)]
                nc.sync.dma_start(
                    out=t.rearrange("p (c w) -> p c w", w=512),
                    in_=xT[:, n * 512:(n + 1) * 512]
                    .rearrange("(c p) w -> p c w", p=128),
                )
                xts_all[n] = views

            load_x(0)
            nc.sync.dma_start(
                out=_qw_tile.rearrange("p (c w) -> p c w", w=EL),
                in_=qw.rearrange("(c p) w -> p c w", p=128),
            )
            nc.sync.dma_start(
                out=vw_all.rearrange("p (c w) -> p c w", w=EL),
                in_=vw.rearrange("(c p) w -> p c w", p=128),
            )
            nc.sync.dma_start(
                out=ctx_all.rearrange("p (c w) -> p c w", w=C)[:, :, 512:1024],
                in_=ctxT[:, 512:1024].rearrange("(c p) w -> p c w", p=128),
            )
            # remaining x tiles up-front: issuing them later would be
            # head-of-line blocked on the SP queue behind the attn transposes
            # (which wait on each tile's last norm)
            for n in range(1, NS):
                load_x(n)
            ow_all = wpool.tile([128, HP * E], F16, name="ow_all")
            ow_sb = [ow_all[:, k * E:(k + 1) * E] for k in range(HP)]
            nc.sync.dma_start(
                out=ow_all.rearrange("p (c w) -> p c w", w=E),
                in_=ow.rearrange("(c p) w -> p c w", p=128),
            )
            ident_sb = wpool.tile([128, 128], F16, name="ident_sb")
            if TR_MODE == "pe":
                nc.sync.dma_start(out=ident_sb[:], in_=ident[:])

            vb_bc = wpool.tile([128, EL], F32, name="vb_bc")

            # ---- static projection targets -------------------------------
            kt_sb = [dpool.tile([128, C], F16, name=f"kt{m}") for m in range(HP)]
            v_sb = [dpool.tile([128, NH * 65], F16, name=f"v{mc}")
                    for mc in range(CC)]

            def kt_thunks(m, ranges=((0, 512), (512, 1024))):
                """Matmul thunks computing K^T column ranges for head pair m."""
                state = {}
                thunks = []

                def f(lo, hi, d):
                    if d == 0:
                        state[lo] = pspool.tile(
                            [128, 512], F32, name="acc_ps", tag="acc", bufs=2)
                    ps = state[lo]
                    nc.tensor.matmul(
                        ps[:, 0:hi - lo],
                        kw_sb[d][:, m * 128:(m + 1) * 128],
                        ctx_sb[d][:, lo:hi],
                        start=(d == 0), stop=(d == KD - 1),
                    )
                    if d == KD - 1:
                        nc.vector.tensor_scalar_add(
                            kt_sb[m][:, lo:hi], ps[:, 0:hi - lo],
                            kbv[:, m, 0:1],
                        )

                for lo, hi in ranges:
                    for d in range(KD):
                        thunks.append((f, lo, hi, d))
                return thunks

            def vproj_thunks(mc):
                """V projection for c-chunk mc, with the ones column."""
                state = {}
                thunks = []

                def f(d):
                    if d == 0:
                        state[0] = pspool.tile(
                            [128, 512], F32, name="acc_ps", tag="acc", bufs=2)
                    ps = state[0]
                    nc.tensor.matmul(
                        ps[:],
                        ctx_sb[d][:, mc * 128:(mc + 1) * 128],
                        vw_sb[d][:],
                        start=(d == 0), stop=(d == KD - 1),
                    )
                    if d == KD - 1:
                        vv = v_sb[mc].rearrange("p (h u) -> p h u", u=65)
                        nc.vector.tensor_add(
                            vv[:, :, 0:64],
                            ps.rearrange("p (h u) -> p h u", u=64),
                            vb_bc.rearrange("p (h u) -> p h u", u=64),
                        )
                        nc.vector.tensor_scalar(
                            vv[:, :, 64:65],
                            vb_bc[:, 0:NH].rearrange("p (h u) -> p h u", u=1),
                            0.0, 1.0,
                            mybir.AluOpType.mult, mybir.AluOpType.add,
                        )  # writes the constant 1.0 column

                for d in range(KD):
                    thunks.append((f, d))
                return thunks

            qts_all = {}

            def qproj_thunks(n, ms=range(HP)):
                """Matmul thunks computing Q^T for s-tile n."""
                state = {}
                thunks = []
                if n not in qts_all:
                    qts_all[n] = [None] * HP

                def f(m, k):
                    if k == 0:
                        state[m] = pspool.tile(
                            [128, 512], F32, name="acc_ps", tag="acc", bufs=2)
                    ps = state[m]
                    nc.tensor.matmul(
                        ps[:],
                        qw_sb[k][:, m * 128:(m + 1) * 128],
                        xts_all[n][k][:],
                        start=(k == 0), stop=(k == KE - 1),
                    )
                    if k == KE - 1:
                        qt_t = qtpool.tile([128, 512], F16, name="qt", tag="qt")
                        nc.vector.tensor_scalar_add(qt_t[:], ps[:],
                                                    kbv[:, m, 1:2])
                        qts_all[n][m] = qt_t

                for m in ms:
                    for k in range(KE):
                        thunks.append((f, m, k))
                return thunks

            attnT_all = {}
            ohalf_all = {}

            def outproj_thunks(n, stage=None):
                """Out-projection of s-tile n from attnT (fp16, [el, s]).

                stage=None: full k=0..3 accumulation, copy, store.
                stage=0: k=0,1 partial into an f32 sbuf tile (needs only
                         heads 0-3 transposed).
                stage=1: k=2 accumulated into the partial (needs heads 4,5).
                stage=2: k=3 + add partial + store (the epilogue remainder).
                """
                state = {}
                thunks = []
                ks = {None: tuple(range(HP)), 0: (0, 1), 1: (2,), 2: (3,),
                      "tail": (2, 3)}[stage]
                k0, k1 = ks[0], ks[-1]
                if stage == 0:
                    ohalf_all[n] = [None] * 4

                def f(ss, ne, hp):
                    if hp == k0:
                        state[(ss, ne)] = pspool.tile(
                            [128, 512], F32, name="acc_ps", tag="acc", bufs=2)
                        if ne == 0:
                            if stage == 0:
                                ohalf_all[n][ss] = opool.tile(
                                    [128, 1024], F32, name="oh_sb", tag="oh",
                                    bufs=4)
                            elif stage != 1:
                                state[ss] = opool.tile(
                                    [128, 1024], F16, name="o_sb", tag="o",
                                    bufs=2)
                    ps = state[(ss, ne)]
                    nc.tensor.matmul(
                        ps[:],
                        attnT_all[n][ss][:, hp * 128:(hp + 1) * 128],
                        ow_sb[hp][:, ne * 512:(ne + 1) * 512],
                        start=(hp == k0), stop=(hp == k1),
                    )
                    if hp == k1:
                        oh = ohalf_all[n][ss][:, ne * 512:(ne + 1) * 512] \
                            if stage is not None else None
                        if stage == 0:
                            nc.vector.tensor_copy(oh, ps[:])
                            return
                        if stage == 1:
                            nc.vector.tensor_add(oh, oh, ps[:])
                            return
                        o_sb = state[ss]
                        if stage in (2, "tail"):
                            nc.vector.tensor_add(
                                o_sb[:, ne * 512:(ne + 1) * 512], oh, ps[:])
                        else:
                            nc.vector.tensor_copy(
                                o_sb[:, ne * 512:(ne + 1) * 512], ps[:])
                        if ne == 1:
                            nc.sync.dma_start(
                                out=out[n * 512 + ss * 128:
                                        n * 512 + (ss + 1) * 128, :],
                                in_=o_sb[:],
                            )

                for ss in range(4):
                    for ne in range(2):
                        for hp in ks:
                            thunks.append((f, ss, ne, hp))
                return thunks

            def run_thunks(ts):
                for f, *args in ts:
                    f(*args)

            # ---- attention for one s-tile --------------------------------
            # Flattened (head, c-pair) steps, with the AV matmuls of step
            # k-1 emitted after the scores matmuls of step k so the exp
            # latency stays off the PE's critical path.  `fg_extra[step]`
            # holds must-run-now foreground work (V projection during s-tile
            # 0); `bg` is paced evenly across the steps.
            def attention(n, bg, early_tr=False):
                # bg: list of (earliest, deadline, thunk) — thunk runs no
                # earlier than step `earliest`, no later than step `deadline`;
                # otherwise paced uniformly across the 32 steps.
                qts = qts_all[n]
                attn_t = [apool.tile([128, 512], F16, name="attn", tag="attn")
                          for _ in range(4)]
                attnT = [atpool.tile([128, 512], F16, name="attnT", tag="attnT")
                         for _ in range(4)]
                attnT_all[n] = attnT
                steps = [(h, cp) for h in range(NH) for cp in range(4)]
                n_steps = len(steps)
                bg_pend = list(bg)
                bg_done = 0
                acc = {}
                pts = {}

                def emit_transpose(lo, hi):
                    for ss in range(4):
                        if TR_MODE == "dma":
                            nc.sync.dma_start_transpose(
                                out=attnT[ss][:, lo:hi]
                                .rearrange("p (k s) -> p k s", s=128),
                                in_=attn_t[ss][:, lo:hi],
                            )
                        else:
                            for k in range(lo // 128, hi // 128):
                                tr_ps = pspool.tile(
                                    [128, 128], F16, name="tr_ps",
                                    tag="tr", bufs=1)
                                nc.tensor.transpose(
                                    tr_ps[:],
                                    attn_t[ss][:, k * 128:(k + 1) * 128],
                                    ident_sb[:])
                                nc.vector.tensor_copy(
                                    attnT[ss][:, k * 128:(k + 1) * 128],
                                    tr_ps[:])

                def emit_av(h, cp):
                    a = acc[h]
                    p_pair = pts[(h, cp)]
                    for ch in range(2):
                        c = 2 * cp + ch
                        for ss in range(4):
                            if EXPW == 1024:
                                p_slice = p_pair[0][:, ch * 512 + ss * 128:
                                                    ch * 512 + (ss + 1) * 128]
                            else:
                                p_slice = p_pair[ch][:, ss * 128:(ss + 1) * 128]
                            nc.tensor.matmul(
                                a[:, ss * 65:(ss + 1) * 65],
                                p_slice,
                                v_sb[c][:, h * 65:(h + 1) * 65],
                                start=(cp == 0 and ch == 0 and ss == 0),
                                stop=(cp == 3 and ch == 1 and ss == 3),
                                skip_group_check=True,
                            )
                    del pts[(h, cp)]

                def emit_norm(h):
                    a = acc.pop(h).rearrange("p (s u) -> p s u", u=65)
                    recip = spool.tile([128, 4], F32, name="rs", tag="rs")
                    with nc.allow_low_precision("softmax denom"):
                        nc.vector.reciprocal(
                            recip.rearrange("p (s u) -> p s u", u=1),
                            a[:, :, 64:65])
                    for ss in range(4):
                        nc.vector.tensor_scalar_mul(
                            attn_t[ss][:, h * 64:(h + 1) * 64],
                            a[:, ss, 0:64],
                            recip[:, ss:ss + 1],
                        )

                for step, (h, cp) in enumerate(steps):
                    pair, h2 = h // 2, h % 2
                    if cp == 0:
                        acc[h] = pspool.tile([128, 260], F32, name="av_ps",
                                             tag="av", bufs=2)
                    if EXPW == 1024:
                        sc = pspool.tile([128, 1024], F32, name="sc_ps",
                                         tag="sc", bufs=sc_bufs)
                        for ch in range(2):
                            c = 2 * cp + ch
                            nc.tensor.matmul(
                                sc[:, ch * 512:(ch + 1) * 512],
                                kt_sb[pair][h2 * 64:(h2 + 1) * 64,
                                            c * 128:(c + 1) * 128],
                                qts[pair][h2 * 64:(h2 + 1) * 64, :],
                                start=True, stop=True,
                            )
                        p_t = ptpool.tile([128, 1024], F16, name="pt",
                                          tag="pt", bufs=3)
                        nc.scalar.activation(p_t[:], sc[:], Exp)
                        pts[(h, cp)] = (p_t,)
                    else:
                        ps = []
                        for ch in range(2):
                            c = 2 * cp + ch
                            sc = pspool.tile([128, 512], F32, name="sc_ps",
                                             tag="sc", bufs=sc_bufs)
                            nc.tensor.matmul(
                                sc[:],
                                kt_sb[pair][h2 * 64:(h2 + 1) * 64,
                                            c * 128:(c + 1) * 128],
                                qts[pair][h2 * 64:(h2 + 1) * 64, :],
                                start=True, stop=True,
                            )
                            p_t = ptpool.tile([128, 512], F16, name="pt",
                                              tag="pt", bufs=6)
                            nc.scalar.activation(p_t[:], sc[:], Exp)
                            ps.append(p_t)
                        pts[(h, cp)] = tuple(ps)

                    # paced background injection (covers the exp latency)
                    quota = (step + 1) * len(bg) // n_steps
                    for it in list(bg_pend):
                        if it[1] <= step:
                            bg_pend.remove(it)
                            f, *args = it[2]
                            f(*args)
                            bg_done += 1
                    while bg_done < quota:
                        for it in bg_pend:
                            if it[0] <= step:
                                bg_pend.remove(it)
                                f, *args = it[2]
                                f(*args)
                                bg_done += 1
                                break
                        else:
                            break

                    # AV matmuls of the previous step (its exp has finished)
                    if step > 0:
                        ph, pcp = steps[step - 1]
                        emit_av(ph, pcp)
                        if pcp == 3:
                            emit_norm(ph)
                            if early_tr and ph == 3:
                                # heads 0-3 done: transpose the first el-half
                                # so the stage-0 out-projection (in bg) can
                                # run before the tile ends
                                emit_transpose(0, 256)

                for it in bg_pend:
                    f, *args = it[2]
                    f(*args)
                bg_pend.clear()
                emit_av(*steps[-1])
                emit_norm(steps[-1][0])
                if early_tr:
                    emit_transpose(256, 512)
                else:
                    emit_transpose(0, 512)

            # ---- prologue ------------------------------------------------
            # PE order: K-proj pair 0 (first data to land), vb broadcast,
            # V-proj chunks 0-3 (fills the PE while xT0/qw stream in),
            # Q-proj m0, then attention(0).  The rest (kt pair0 second half,
            # V-proj 4-7, K/Q-proj of the other pairs, Q-proj of s-tile 1)
            # rides s-tile 0's background, front-boosted so the
            # early-deadline items land before the steps that consume them.
            run_thunks(kt_thunks(0, ranges=((0, 512),)))
            vb_ps = pspool.tile([128, 512], F32, name="acc_ps", tag="acc", bufs=2)
            nc.tensor.matmul(vb_ps[:], ones_sb[0:1, :], vb_sb[:],
                             start=True, stop=True)
            nc.vector.tensor_copy(vb_bc[:], vb_ps[:])
            qp0 = qproj_thunks(0)
            run_thunks(qp0[:KE])                       # m=0 group

            # Background scheduling: each item gets an (earliest, deadline)
            # step window.  Q-proj of tile n+1 is split — m0 completes in
            # tile n, m1-3 ride tile n+1 itself with deadlines just before
            # the heads that consume them.  This smooths s-tile 0's PE
            # overload (K/V/Q projections) into the ACT slack of the
            # later tiles.
            for n in range(NS):
                bg = []
                if n == 0:
                    for mc in range(CC):
                        bg += [(0, mc // 2, t) for t in vproj_thunks(mc)]
                    bg += [(0, 1, t) for t in kt_thunks(0, ranges=((512, 1024),))]
                    for m in range(1, HP):
                        bg += [(0, 8 * m - 2, t) for t in kt_thunks(m)]
                        bg += [(0, 8 * m - 2, t)
                               for t in qp0[m * KE:(m + 1) * KE]]
                else:
                    for m in range(1, HP):
                        bg += [(0, 8 * m - 2, t)
                               for t in qproj_thunks(n, ms=(m,))]
                    bg += [(0, 31, t) for t in outproj_thunks(n - 1)]
                if n + 1 < NS:
                    bg += [(0, 31, t) for t in qproj_thunks(n + 1, ms=(0,))]
                if n == NS - 1:
                    bg += [(17, 31, t) for t in outproj_thunks(n, stage=0)]
                attention(n, bg, early_tr=(n == NS - 1))

            # epilogue: remaining el-chunks of the last out-projection
            run_thunks(outproj_thunks(NS - 1, stage="tail"))

    nc.finalize()
    return nc


def kernel(x, context, q_w, q_b, k_w, k_b, v_w, v_b, o_w, o_b):
    global _built, _last_results
    from concourse.bass_utils import run_bass_kernel_spmd

    if _built is None:
        _built = _build()
    nc = _built

    scale = np.float32(1.0 / np.sqrt(HD))
    f16 = np.float16
    x = np.asarray(x, np.float32)
    context = np.asarray(context, np.float32)
    xTs = [np.ascontiguousarray(x[b].T).astype(f16) for b in range(B)]
    ctxTs = [np.ascontiguousarray(context[b].T).astype(f16) for b in range(B)]

    in_maps = []
    for core in range(N_CORES):
        b, hg = core // 2, core % 2
        el = slice(hg * EL, (hg + 1) * EL)
        in_maps.append({
            "xT": xTs[b],
            "ctxT": ctxTs[b],
            "qw": np.ascontiguousarray(
                (np.asarray(q_w, np.float32)[:, el] * scale)).astype(f16),
            "kw": np.ascontiguousarray(np.asarray(k_w, np.float32)[:, el]).astype(f16),
            "vw": np.ascontiguousarray(np.asarray(v_w, np.float32)[:, el]).astype(f16),
            "ow": np.ascontiguousarray(np.asarray(o_w, np.float32)[el, :]).astype(f16),
            "kbqb": np.ascontiguousarray(np.stack(
                [np.asarray(k_b, np.float32)[el],
                 np.asarray(q_b, np.float32)[el] * scale], axis=1)),
            "vbo": np.ascontiguousarray(np.concatenate(
                [np.asarray(v_b, np.float32)[el].astype(f16),
                 np.ones(128, f16)])[None, :]),
            "ident": np.eye(128, dtype=f16),
        })

    res = run_bass_kernel_spmd(nc, in_maps, list(range(N_CORES)))
    _last_results = res

    ob = np.asarray(o_b, np.float32)
    full = np.empty((B, S, E), np.float32)
    for b in range(B):
        full[b] = (res.results[2 * b]["out"].astype(np.float32)
                   + res.results[2 * b + 1]["out"].astype(np.float32) + ob)
    return full
